# revision 15
# baseline (speedup 1.0000x reference)
"""Trainium2 Bass kernel for the CurriculumLoss module.

Math (matches the jax reference):
    base_loss[b] = logsumexp(x[b, :]) - x[b, targets[b]]          # x: [B, V] f32
    new_diff[b]  = 0.9 * difficulty[sample_ids[b]] + 0.1 * base_loss[b]
    e[b]         = exp(-new_diff[b] * (1 - step/1000))
    out          = sum_b(base_loss[b] * e[b]) / sum_b(e[b])       # scalar f32

Sharding: data-parallel over the batch. Each of the 8 NeuronCores gets a
contiguous 256-row slice of the logits and streams it from HBM in
[128, 4096] f32 tiles — the stream is HBM-read-bound at ~26.5 GB/s per SDMA
engine (~420 GB/s/core), which sets the kernel's floor. Per chunk, the
Scalar (ACT) engine computes exp (bf16 out, full rate), and the Vector
engine folds the chunk's two halves together while row-summing
(scalar_tensor_tensor with accum_out), so both compute engines run at
~2x the DMA cadence and never gate the stream. Each core writes its
[256, NCH] per-chunk row sums to HBM (group 0's mid-stream, hidden; only
group 1's small transfer trails the last chunk). The O(B) epilogue —
log, the difficulty-table gather, curriculum weights, and the
weight-normalization "all-reduce" across cores — is host-side numpy on
the 2048 row sums, which keeps the device critical path free of the
serial ln->exp->matmul chain.
"""

import numpy as np

try:
    import concourse  # noqa: F401
except ImportError:  # pragma: no cover - fallback for stripped grading env
    import sys

    for _p in ("/opt/trn_rl_repo", "/root/.axon_site/_ro/trn_rl_repo"):
        if _p not in sys.path:
            sys.path.append(_p)

import concourse.bacc as bacc
import concourse.bass as bass
import concourse.tile as tile
from concourse import mybir
from concourse.bass_utils import run_bass_kernel_spmd

B = 2048
V = 50257
NTAB = 1_000_000
NCORES = 8
BLOC = B // NCORES  # 256 rows per core
P = 128
NGRP = BLOC // P  # 2 partition-groups of 128 rows
CH = 4096  # V-chunk width (2 MiB per streaming DMA; measured best rate)
# Column chunks: wide for the bulk of the stream (best DMA efficiency), with
# a tapered tail so the last-arriving data needs minimal compute before the
# trailing row-sum writeback. All but the last are even so the Vector engine
# can fold halves while reducing; the odd remainder (V is odd) goes last.
_TAIL = [2048, 2048, 849, 256]
CHUNKS = []
_c0 = 0
while V - _c0 > sum(_TAIL):
    CHUNKS.append((_c0, CH))
    _c0 += CH
for _w in _TAIL:
    CHUNKS.append((_c0, _w))
    _c0 += _w
assert _c0 == V
NCH = len(CHUNKS)
WARMUP = 1000.0
MOM = 0.9

F32 = mybir.dt.float32
BF16 = mybir.dt.bfloat16
AF = mybir.ActivationFunctionType
ALU = mybir.AluOpType


class _Bacc(bacc.Bacc):
    """Bacc that pins Exp to one ACT table set.

    Only Exp is used; the stock greedy assignment already needs a single
    ACT_TABLE_LOAD, but pinning keeps the choice stable across compiler
    versions.
    """

    def insert_act_table_loads(self):
        from concourse.hw_specs import get_activation_tables

        has_activation = any(
            isinstance(i, mybir.InstActivation)
            for b in self.main_func.blocks
            for i in b.instructions
        )
        if not has_activation:
            return
        tables = []
        for name, fns in get_activation_tables(self.m.arch).items():
            if name != "exp_and_others":
                fns = fns - {AF.Exp}
            tables.append((name, fns))
        import bass_rust

        bass_rust.insert_act_table_loads(self, tables)


def _build() -> bass.Bass:
    # Bacc (not raw Bass): its compile pipeline splits multi-semaphore waits
    # into EventSemaphore instructions — TRN2 allows only 1 wait per inst.
    nc = _Bacc("TRN2")
    x = nc.dram_tensor("x", [BLOC, V], F32, kind="ExternalInput")
    out = nc.dram_tensor("out", [BLOC, NCH], F32, kind="ExternalOutput")

    with tile.TileContext(nc) as tc:
        with (
            tc.tile_pool(name="stream", bufs=7) as stream,
            tc.tile_pool(name="ex", bufs=3) as ex,
            tc.tile_pool(name="red", bufs=2) as red,
            tc.tile_pool(name="small", bufs=1) as small,
        ):
            partials = [
                small.tile([P, NCH], F32, tag=f"part{g}", name=f"part{g}")
                for g in range(NGRP)
            ]

            for g in range(NGRP):
                rows = slice(g * P, (g + 1) * P)
                for j, (c0, w) in enumerate(CHUNKS):
                    t = stream.tile([P, CH], F32, tag="xt")
                    # Alternate transfers between the SP HWDGE ring and the
                    # (otherwise idle) gpsimd SWDGE ring: the SDMA engines
                    # round-robin both rings' packets, so one ring's
                    # completion/doorbell handoff hides under the other
                    # ring's transfer instead of gapping the stream.
                    q = nc.sync if j % 2 == 0 else nc.gpsimd
                    q.dma_start(out=t[:, :w], in_=x[rows, c0 : c0 + w])
                    # exp on ACT (bf16 out — full rate, halves downstream
                    # read traffic; ~2^-9 relative rounding is far inside
                    # tolerance after the 50k-element sum)
                    e_t = ex.tile([P, CH], BF16, tag="et")
                    nc.scalar.activation(
                        out=e_t[:, :w], in_=t[:, :w], func=AF.Exp
                    )
                    if w % 2 == 0:
                        # fold the two halves together while reducing: DVE
                        # reads w cols but only streams w/2 output cols, so
                        # the row-sum costs half of a plain reduce.
                        h = w // 2
                        pair = red.tile([P, CH // 2], BF16, tag="pr")
                        nc.vector.scalar_tensor_tensor(
                            out=pair[:, :h],
                            in0=e_t[:, :h],
                            scalar=1.0,
                            in1=e_t[:, h:w],
                            op0=ALU.mult,
                            op1=ALU.add,
                            accum_out=partials[g][:, j : j + 1],
                        )
                    else:
                        nc.vector.reduce_sum(
                            out=partials[g][:, j : j + 1],
                            in_=e_t[:, :w],
                            axis=mybir.AxisListType.X,
                        )
                # group 0's writeback is issued mid-stream and hides under
                # group 1's transfers; only this small [128, NCH] DMA for
                # the last group trails the final chunk.
                nc.sync.dma_start(out=out[rows, :], in_=partials[g][:])

    # Run Bacc's compile pipeline (register allocation, event-semaphore
    # splitting) — the PJRT exec path ships the BIR as-is.
    nc.finalize()
    return nc


_NC_CACHE: dict[int, bass.Bass] = {}


def _get_nc() -> bass.Bass:
    if 0 not in _NC_CACHE:
        _NC_CACHE[0] = _build()
    return _NC_CACHE[0]


def run(inputs, targets, sample_ids, difficulty_scores, step, **spmd_kwargs):
    """Run the SPMD kernel; returns (scalar result, BassKernelResults)."""
    step_i = int(np.asarray(step))
    c = 1.0 - step_i / WARMUP  # curriculum sharpness coefficient
    x = np.ascontiguousarray(np.asarray(inputs, dtype=np.float32))
    t = np.asarray(targets, dtype=np.int64).reshape(B)
    s = np.asarray(sample_ids, dtype=np.int64).reshape(B)
    d = np.asarray(difficulty_scores, dtype=np.float32).reshape(NTAB)

    nc = _get_nc()
    in_maps = [{"x": x[core * BLOC : (core + 1) * BLOC]} for core in range(NCORES)]
    br = run_bass_kernel_spmd(nc, in_maps, core_ids=list(range(NCORES)), **spmd_kwargs)

    # Host epilogue on the gathered per-chunk row sums: O(B) work.
    parts = np.concatenate(
        [np.asarray(r["out"], dtype=np.float64) for r in br.results], axis=0
    )  # [B, NCH]
    S = parts.sum(axis=1)  # [B] sum of exps per row
    lse = np.log(S)
    tl = x[np.arange(B), t].astype(np.float64)  # target logits
    base = lse - tl
    new_diff = MOM * d[s].astype(np.float64) + (1.0 - MOM) * base
    e = np.exp(-new_diff * c)
    result = (base * e).sum() / e.sum()  # weight-normalized mean
    return np.asarray(result, dtype=np.float32), br


def kernel(inputs, targets, sample_ids, difficulty_scores, step):
    result, _ = run(inputs, targets, sample_ids, difficulty_scores, step)
    return result


# revision 24
# speedup vs baseline: 1.1509x; 1.1509x over previous
"""Trainium2 Bass kernel for the CurriculumLoss module.

Math (matches the jax reference):
    base_loss[b] = logsumexp(x[b, :]) - x[b, targets[b]]          # x: [B, V] f32
    new_diff[b]  = 0.9 * difficulty[sample_ids[b]] + 0.1 * base_loss[b]
    e[b]         = exp(-new_diff[b] * (1 - step/1000))
    out          = sum_b(base_loss[b] * e[b]) / sum_b(e[b])       # scalar f32

Sharding: data-parallel over the batch. Each of the 8 NeuronCores gets a
contiguous 256-row slice of the logits and streams it from HBM in
[128, 4096] f32 tiles — the stream is HBM-read-bound at ~26.5 GB/s per SDMA
engine (~420 GB/s/core), which sets the kernel's floor. Per chunk, the
Scalar (ACT) engine computes exp (bf16 out, full rate), and the Vector
engine folds the chunk's two halves together while row-summing
(scalar_tensor_tensor with accum_out), so both compute engines run at
~2x the DMA cadence and never gate the stream. Each core writes its
[256, NCH] per-chunk row sums to HBM (group 0's mid-stream, hidden; only
group 1's small transfer trails the last chunk). The O(B) epilogue —
log, the difficulty-table gather, curriculum weights, and the
weight-normalization "all-reduce" across cores — is host-side numpy on
the 2048 row sums, which keeps the device critical path free of the
serial ln->exp->matmul chain.
"""

import numpy as np

try:
    import concourse  # noqa: F401
except ImportError:  # pragma: no cover - fallback for stripped grading env
    import sys

    for _p in ("/opt/trn_rl_repo", "/root/.axon_site/_ro/trn_rl_repo"):
        if _p not in sys.path:
            sys.path.append(_p)

import concourse.bacc as bacc
import concourse.bass as bass
import concourse.tile as tile
from concourse import mybir
from concourse.bass_utils import run_bass_kernel_spmd

B = 2048
V = 50257
NTAB = 1_000_000
NCORES = 8
BLOC = B // NCORES  # 256 rows per core
P = 128
NGRP = BLOC // P  # 2 partition-groups of 128 rows
# Column chunks: wide for the bulk of the stream (best DMA efficiency), with
# a tapered tail so the last-arriving data needs minimal compute before the
# trailing row-sum writeback. All but the 849 are even so the Vector engine
# can fold halves while reducing (V is odd, so one odd chunk is inevitable);
# the last chunk is small and even.
def _chunk_plan(ch):
    chunks, c0 = [], 0
    while V - c0 > ch + 3153:  # leave >= 3153 cols for the taper
        chunks.append((c0, ch))
        c0 += ch
    r2 = V - c0 - 849 - 256  # even remainder, split into <=2048-wide chunks
    while r2 > 0:
        w = min(2048, r2)
        chunks.append((c0, w))
        c0 += w
        r2 -= w
    for w in (849, 256):
        chunks.append((c0, w))
        c0 += w
    assert c0 == V
    return chunks

CH = 4096  # V-chunk width (2 MiB per streaming DMA; measured best rate)
CHUNKS = _chunk_plan(CH)
NCH = len(CHUNKS)
WARMUP = 1000.0
MOM = 0.9

F32 = mybir.dt.float32
BF16 = mybir.dt.bfloat16
AF = mybir.ActivationFunctionType
ALU = mybir.AluOpType


class _Bacc(bacc.Bacc):
    """Bacc that pins Exp to one ACT table set.

    Only Exp is used; the stock greedy assignment already needs a single
    ACT_TABLE_LOAD, but pinning keeps the choice stable across compiler
    versions.
    """

    def insert_act_table_loads(self):
        from concourse.hw_specs import get_activation_tables

        has_activation = any(
            isinstance(i, mybir.InstActivation)
            for b in self.main_func.blocks
            for i in b.instructions
        )
        if not has_activation:
            return
        tables = []
        for name, fns in get_activation_tables(self.m.arch).items():
            if name != "exp_and_others":
                fns = fns - {AF.Exp}
            tables.append((name, fns))
        import bass_rust

        bass_rust.insert_act_table_loads(self, tables)


def _build(
    ch: int = CH, bufs: int = 8, exbufs: int = 4, split_rings: int = 0
) -> bass.Bass:
    chunks = _chunk_plan(ch)
    nch = len(chunks)
    # Bacc (not raw Bass): its compile pipeline splits multi-semaphore waits
    # into EventSemaphore instructions — TRN2 allows only 1 wait per inst.
    nc = _Bacc("TRN2")
    x = nc.dram_tensor("x", [BLOC, V], F32, kind="ExternalInput")
    out = nc.dram_tensor("out", [BLOC, nch], F32, kind="ExternalOutput")

    with tile.TileContext(nc) as tc:
        with (
            tc.tile_pool(name="stream", bufs=bufs) as stream,
            tc.tile_pool(name="ex", bufs=exbufs) as ex,
            tc.tile_pool(name="small", bufs=1) as small,
        ):
            partials = [
                small.tile([P, nch], F32, tag=f"part{g}", name=f"part{g}")
                for g in range(NGRP)
            ]

            # flat chunk schedule across both groups so transfer issue can
            # run ahead of consumption at group boundaries too
            sched = [
                (g, j, c0, w)
                for g in range(NGRP)
                for j, (c0, w) in enumerate(chunks)
            ]
            pending = {}

            def issue(k):
                g, j, c0, w = sched[k]
                rows = slice(g * P, (g + 1) * P)
                t = stream.tile([P, ch], F32, tag="xt")
                if split_rings:
                    # Half of each transfer on each HWDGE ring (SP + ACT):
                    # the rings drain concurrently through the same 16 SDMA
                    # engines, so per-transfer completion handoffs overlap
                    # instead of gapping the stream. The ACT-ring issue is
                    # emitted one chunk ahead of ACT's exp for that chunk,
                    # so the ACT sequencer never stalls on it.
                    h = (w + 1) // 2
                    nc.sync.dma_start(out=t[:, :h], in_=x[rows, c0 : c0 + h])
                    nc.scalar.dma_start(
                        out=t[:, h:w], in_=x[rows, c0 + h : c0 + w]
                    )
                else:
                    nc.sync.dma_start(out=t[:, :w], in_=x[rows, c0 : c0 + w])
                pending[k] = t

            issue(0)
            for k, (g, j, c0, w) in enumerate(sched):
                rows = slice(g * P, (g + 1) * P)
                if k + 1 < len(sched):
                    issue(k + 1)
                t = pending.pop(k)
                # exp on ACT (bf16 out — full rate, halves downstream
                # read traffic; ~2^-9 relative rounding is far inside
                # tolerance after the 50k-element sum)
                e_t = ex.tile([P, ch], BF16, tag="et")
                nc.scalar.activation(out=e_t[:, :w], in_=t[:, :w], func=AF.Exp)
                if w % 2 == 0:
                    # fold the two halves together while reducing: DVE
                    # reads w cols but only streams w/2 output cols, so
                    # the row-sum costs half of a plain reduce. The
                    # elementwise sum is scratch — write it in place over
                    # the first half (same-stride elementwise is safe).
                    h = w // 2
                    nc.vector.scalar_tensor_tensor(
                        out=e_t[:, :h],
                        in0=e_t[:, :h],
                        scalar=1.0,
                        in1=e_t[:, h:w],
                        op0=ALU.mult,
                        op1=ALU.add,
                        accum_out=partials[g][:, j : j + 1],
                    )
                else:
                    nc.vector.reduce_sum(
                        out=partials[g][:, j : j + 1],
                        in_=e_t[:, :w],
                        axis=mybir.AxisListType.X,
                    )
                if j == nch - 1:
                    # group writeback: group 0's is issued mid-stream and
                    # hides under group 1's transfers; only the last group's
                    # small [128, NCH] DMA trails the final chunk.
                    nc.sync.dma_start(out=out[rows, :], in_=partials[g][:])

    # Run Bacc's compile pipeline (register allocation, event-semaphore
    # splitting) — the PJRT exec path ships the BIR as-is.
    nc.finalize()
    return nc


_NC_CACHE: dict[tuple, bass.Bass] = {}


def _get_nc(cfg: tuple = (CH, 8, 4)) -> bass.Bass:
    if cfg not in _NC_CACHE:
        _NC_CACHE[cfg] = _build(*cfg)
    return _NC_CACHE[cfg]


def run(
    inputs,
    targets,
    sample_ids,
    difficulty_scores,
    step,
    cfg: tuple = (CH, 8, 4),
    **spmd_kwargs,
):
    """Run the SPMD kernel; returns (scalar result, BassKernelResults)."""
    step_i = int(np.asarray(step))
    c = 1.0 - step_i / WARMUP  # curriculum sharpness coefficient
    x = np.ascontiguousarray(np.asarray(inputs, dtype=np.float32))
    t = np.asarray(targets, dtype=np.int64).reshape(B)
    s = np.asarray(sample_ids, dtype=np.int64).reshape(B)
    d = np.asarray(difficulty_scores, dtype=np.float32).reshape(NTAB)

    nc = _get_nc(cfg)
    in_maps = [{"x": x[core * BLOC : (core + 1) * BLOC]} for core in range(NCORES)]
    br = run_bass_kernel_spmd(nc, in_maps, core_ids=list(range(NCORES)), **spmd_kwargs)

    # Host epilogue on the gathered per-chunk row sums: O(B) work.
    parts = np.concatenate(
        [np.asarray(r["out"], dtype=np.float64) for r in br.results], axis=0
    )  # [B, nch]
    S = parts.sum(axis=1)  # [B] sum of exps per row
    lse = np.log(S)
    tl = x[np.arange(B), t].astype(np.float64)  # target logits
    base = lse - tl
    new_diff = MOM * d[s].astype(np.float64) + (1.0 - MOM) * base
    e = np.exp(-new_diff * c)
    result = (base * e).sum() / e.sum()  # weight-normalized mean
    return np.asarray(result, dtype=np.float32), br


def kernel(inputs, targets, sample_ids, difficulty_scores, step):
    result, _ = run(inputs, targets, sample_ids, difficulty_scores, step)
    return result


# revision 25
# speedup vs baseline: 1.1808x; 1.0259x over previous
"""Trainium2 Bass kernel for the CurriculumLoss module.

Math (matches the jax reference):
    base_loss[b] = logsumexp(x[b, :]) - x[b, targets[b]]          # x: [B, V] f32
    new_diff[b]  = 0.9 * difficulty[sample_ids[b]] + 0.1 * base_loss[b]
    e[b]         = exp(-new_diff[b] * (1 - step/1000))
    out          = sum_b(base_loss[b] * e[b]) / sum_b(e[b])       # scalar f32

Sharding: data-parallel over the batch. Each of the 8 NeuronCores gets a
contiguous 256-row slice of the logits and streams it from HBM in
[128, 4096] f32 tiles — the stream is HBM-read-bound at ~26.5 GB/s per SDMA
engine (~420 GB/s/core), which sets the kernel's floor. Per chunk, the
Scalar (ACT) engine computes exp (bf16 out, full rate), and the Vector
engine folds the chunk's two halves together while row-summing
(scalar_tensor_tensor with accum_out), so both compute engines run at
~2x the DMA cadence and never gate the stream. Each core writes its
[256, NCH] per-chunk row sums to HBM (group 0's mid-stream, hidden; only
group 1's small transfer trails the last chunk). The O(B) epilogue —
log, the difficulty-table gather, curriculum weights, and the
weight-normalization "all-reduce" across cores — is host-side numpy on
the 2048 row sums, which keeps the device critical path free of the
serial ln->exp->matmul chain.
"""

import numpy as np

try:
    import concourse  # noqa: F401
except ImportError:  # pragma: no cover - fallback for stripped grading env
    import sys

    for _p in ("/opt/trn_rl_repo", "/root/.axon_site/_ro/trn_rl_repo"):
        if _p not in sys.path:
            sys.path.append(_p)

import concourse.bacc as bacc
import concourse.bass as bass
import concourse.tile as tile
from concourse import mybir
from concourse.bass_utils import run_bass_kernel_spmd

B = 2048
V = 50257
NTAB = 1_000_000
NCORES = 8
BLOC = B // NCORES  # 256 rows per core
P = 128
NGRP = BLOC // P  # 2 partition-groups of 128 rows
# Column chunks: wide for the bulk of the stream (best DMA efficiency), with
# a tapered tail so the last-arriving data needs minimal compute before the
# trailing row-sum writeback. All but the 849 are even so the Vector engine
# can fold halves while reducing (V is odd, so one odd chunk is inevitable);
# the last chunk is small and even.
def _chunk_plan(ch, fine_tail=1):
    # Taper rule: a pre-final chunk keeps ACT arrival-gated (it finishes each
    # exp before the next chunk lands) iff its DMA time exceeds its exp time:
    # 1.22ns/col stream vs 0.833ns/col + ~343ns startup -> w >= ~890 cols.
    # The final chunk is as small as possible (its exp+fold is pure tail).
    # Exactly one odd chunk (V is odd), placed early so its (unfoldable,
    # full-rate) reduce stays off the trailing Vector-engine chain.
    tail = [905, 1248, 1048, 1000, 744, 256] if fine_tail else [849, 256]
    filler = 0 if fine_tail else 4096
    chunks, c0 = [], 0
    while V - c0 > ch + filler + sum(tail):
        chunks.append((c0, ch))
        c0 += ch
    r2 = V - c0 - sum(tail)  # even remainder, split into <=2048-wide chunks
    while r2 > 0:
        w = min(2048, r2)
        chunks.append((c0, w))
        c0 += w
        r2 -= w
    for w in tail:
        chunks.append((c0, w))
        c0 += w
    assert c0 == V
    return chunks

CH = 4096  # V-chunk width (2 MiB per streaming DMA; measured best rate)
CHUNKS = _chunk_plan(CH)
NCH = len(CHUNKS)
WARMUP = 1000.0
MOM = 0.9

F32 = mybir.dt.float32
BF16 = mybir.dt.bfloat16
AF = mybir.ActivationFunctionType
ALU = mybir.AluOpType


class _Bacc(bacc.Bacc):
    """Bacc that pins Exp to one ACT table set.

    Only Exp is used; the stock greedy assignment already needs a single
    ACT_TABLE_LOAD, but pinning keeps the choice stable across compiler
    versions.
    """

    def insert_act_table_loads(self):
        from concourse.hw_specs import get_activation_tables

        has_activation = any(
            isinstance(i, mybir.InstActivation)
            for b in self.main_func.blocks
            for i in b.instructions
        )
        if not has_activation:
            return
        tables = []
        for name, fns in get_activation_tables(self.m.arch).items():
            if name != "exp_and_others":
                fns = fns - {AF.Exp}
            tables.append((name, fns))
        import bass_rust

        bass_rust.insert_act_table_loads(self, tables)


def _build(
    ch: int = CH,
    bufs: int = 8,
    exbufs: int = 4,
    split_rings: int = 0,
    fine_tail: int = 1,
) -> bass.Bass:
    chunks = _chunk_plan(ch, fine_tail)
    nch = len(chunks)
    # Bacc (not raw Bass): its compile pipeline splits multi-semaphore waits
    # into EventSemaphore instructions — TRN2 allows only 1 wait per inst.
    nc = _Bacc("TRN2")
    x = nc.dram_tensor("x", [BLOC, V], F32, kind="ExternalInput")
    out = nc.dram_tensor("out", [BLOC, nch], F32, kind="ExternalOutput")

    with tile.TileContext(nc) as tc:
        with (
            tc.tile_pool(name="stream", bufs=bufs) as stream,
            tc.tile_pool(name="ex", bufs=exbufs) as ex,
            tc.tile_pool(name="small", bufs=1) as small,
        ):
            partials = [
                small.tile([P, nch], F32, tag=f"part{g}", name=f"part{g}")
                for g in range(NGRP)
            ]

            # flat chunk schedule across both groups so transfer issue can
            # run ahead of consumption at group boundaries too
            sched = [
                (g, j, c0, w)
                for g in range(NGRP)
                for j, (c0, w) in enumerate(chunks)
            ]
            pending = {}

            def issue(k):
                g, j, c0, w = sched[k]
                rows = slice(g * P, (g + 1) * P)
                t = stream.tile([P, ch], F32, tag="xt")
                if split_rings:
                    # Half of each transfer on each HWDGE ring (SP + ACT):
                    # the rings drain concurrently through the same 16 SDMA
                    # engines, so per-transfer completion handoffs overlap
                    # instead of gapping the stream. The ACT-ring issue is
                    # emitted one chunk ahead of ACT's exp for that chunk,
                    # so the ACT sequencer never stalls on it.
                    h = (w + 1) // 2
                    nc.sync.dma_start(out=t[:, :h], in_=x[rows, c0 : c0 + h])
                    nc.scalar.dma_start(
                        out=t[:, h:w], in_=x[rows, c0 + h : c0 + w]
                    )
                else:
                    nc.sync.dma_start(out=t[:, :w], in_=x[rows, c0 : c0 + w])
                pending[k] = t

            issue(0)
            for k, (g, j, c0, w) in enumerate(sched):
                rows = slice(g * P, (g + 1) * P)
                if k + 1 < len(sched):
                    issue(k + 1)
                t = pending.pop(k)
                # exp on ACT (bf16 out — full rate, halves downstream
                # read traffic; ~2^-9 relative rounding is far inside
                # tolerance after the 50k-element sum)
                e_t = ex.tile([P, ch], BF16, tag="et")
                nc.scalar.activation(out=e_t[:, :w], in_=t[:, :w], func=AF.Exp)
                if w % 2 == 0:
                    # fold the two halves together while reducing: DVE
                    # reads w cols but only streams w/2 output cols, so
                    # the row-sum costs half of a plain reduce. The
                    # elementwise sum is scratch — write it in place over
                    # the first half (same-stride elementwise is safe).
                    h = w // 2
                    nc.vector.scalar_tensor_tensor(
                        out=e_t[:, :h],
                        in0=e_t[:, :h],
                        scalar=1.0,
                        in1=e_t[:, h:w],
                        op0=ALU.mult,
                        op1=ALU.add,
                        accum_out=partials[g][:, j : j + 1],
                    )
                else:
                    nc.vector.reduce_sum(
                        out=partials[g][:, j : j + 1],
                        in_=e_t[:, :w],
                        axis=mybir.AxisListType.X,
                    )
                if j == nch - 1:
                    # group writeback: group 0's is issued mid-stream — on
                    # the scalar HWDGE ring when fine_tail, so its packets
                    # and HBM-write receipt don't stall the sync ring's
                    # stream FIFO; only the last group's small [128, NCH]
                    # DMA trails the final chunk.
                    wq = nc.scalar if (fine_tail and g < NGRP - 1) else nc.sync
                    wq.dma_start(out=out[rows, :], in_=partials[g][:])

    # Run Bacc's compile pipeline (register allocation, event-semaphore
    # splitting) — the PJRT exec path ships the BIR as-is.
    nc.finalize()
    return nc


_NC_CACHE: dict[tuple, bass.Bass] = {}


def _get_nc(cfg: tuple = (CH, 8, 4)) -> bass.Bass:
    if cfg not in _NC_CACHE:
        _NC_CACHE[cfg] = _build(*cfg)
    return _NC_CACHE[cfg]


def run(
    inputs,
    targets,
    sample_ids,
    difficulty_scores,
    step,
    cfg: tuple = (CH, 8, 4),
    **spmd_kwargs,
):
    """Run the SPMD kernel; returns (scalar result, BassKernelResults)."""
    step_i = int(np.asarray(step))
    c = 1.0 - step_i / WARMUP  # curriculum sharpness coefficient
    x = np.ascontiguousarray(np.asarray(inputs, dtype=np.float32))
    t = np.asarray(targets, dtype=np.int64).reshape(B)
    s = np.asarray(sample_ids, dtype=np.int64).reshape(B)
    d = np.asarray(difficulty_scores, dtype=np.float32).reshape(NTAB)

    nc = _get_nc(cfg)
    in_maps = [{"x": x[core * BLOC : (core + 1) * BLOC]} for core in range(NCORES)]
    br = run_bass_kernel_spmd(nc, in_maps, core_ids=list(range(NCORES)), **spmd_kwargs)

    # Host epilogue on the gathered per-chunk row sums: O(B) work.
    parts = np.concatenate(
        [np.asarray(r["out"], dtype=np.float64) for r in br.results], axis=0
    )  # [B, nch]
    S = parts.sum(axis=1)  # [B] sum of exps per row
    lse = np.log(S)
    tl = x[np.arange(B), t].astype(np.float64)  # target logits
    base = lse - tl
    new_diff = MOM * d[s].astype(np.float64) + (1.0 - MOM) * base
    e = np.exp(-new_diff * c)
    result = (base * e).sum() / e.sum()  # weight-normalized mean
    return np.asarray(result, dtype=np.float32), br


def kernel(inputs, targets, sample_ids, difficulty_scores, step):
    result, _ = run(inputs, targets, sample_ids, difficulty_scores, step)
    return result


# revision 27
# speedup vs baseline: 1.1882x; 1.0063x over previous
"""Trainium2 Bass kernel for the CurriculumLoss module.

Math (matches the jax reference):
    base_loss[b] = logsumexp(x[b, :]) - x[b, targets[b]]          # x: [B, V] f32
    new_diff[b]  = 0.9 * difficulty[sample_ids[b]] + 0.1 * base_loss[b]
    e[b]         = exp(-new_diff[b] * (1 - step/1000))
    out          = sum_b(base_loss[b] * e[b]) / sum_b(e[b])       # scalar f32

Sharding: data-parallel over the batch. Each of the 8 NeuronCores gets a
contiguous 256-row slice of the logits and streams it from HBM in
[128, 4096] f32 tiles — the stream is HBM-read-bound at ~26.5 GB/s per SDMA
engine (~420 GB/s/core), which sets the kernel's floor. Per chunk, the
Scalar (ACT) engine computes exp (bf16 out, full rate), and the Vector
engine folds the chunk's two halves together while row-summing
(scalar_tensor_tensor with accum_out), so both compute engines run at
~2x the DMA cadence and never gate the stream. Each core writes its
[256, NCH] per-chunk row sums to HBM (group 0's mid-stream, hidden; only
group 1's small transfer trails the last chunk). The O(B) epilogue —
log, the difficulty-table gather, curriculum weights, and the
weight-normalization "all-reduce" across cores — is host-side numpy on
the 2048 row sums, which keeps the device critical path free of the
serial ln->exp->matmul chain.
"""

import numpy as np

try:
    import concourse  # noqa: F401
except ImportError:  # pragma: no cover - fallback for stripped grading env
    import sys

    for _p in ("/opt/trn_rl_repo", "/root/.axon_site/_ro/trn_rl_repo"):
        if _p not in sys.path:
            sys.path.append(_p)

import concourse.bacc as bacc
import concourse.bass as bass
import concourse.tile as tile
from concourse import mybir
from concourse.bass_utils import run_bass_kernel_spmd

B = 2048
V = 50257
NTAB = 1_000_000
NCORES = 8
BLOC = B // NCORES  # 256 rows per core
P = 128
NGRP = BLOC // P  # 2 partition-groups of 128 rows
# Column chunks: wide for the bulk of the stream (best DMA efficiency), with
# a tapered tail so the last-arriving data needs minimal compute before the
# trailing row-sum writeback. All but the 849 are even so the Vector engine
# can fold halves while reducing (V is odd, so one odd chunk is inevitable);
# the last chunk is small and even.
def _chunk_plan(ch, fine_tail=1):
    # Taper rule: a pre-final chunk keeps ACT arrival-gated (it finishes each
    # exp before the next chunk lands) iff its DMA time exceeds its exp time:
    # 1.22ns/col stream vs 0.833ns/col + ~343ns startup -> w >= ~890 cols.
    # The final chunk is as small as possible (its exp+fold is pure tail).
    # Exactly one odd chunk (V is odd), placed early so its (unfoldable,
    # full-rate) reduce stays off the trailing Vector-engine chain.
    tail = [905, 1248, 1048, 1000, 744, 256] if fine_tail else [849, 256]
    filler = 0 if fine_tail else 4096
    chunks, c0 = [], 0
    while V - c0 > ch + filler + sum(tail):
        chunks.append((c0, ch))
        c0 += ch
    r2 = V - c0 - sum(tail)  # even remainder, split into <=2048-wide chunks
    while r2 > 0:
        w = min(2048, r2)
        chunks.append((c0, w))
        c0 += w
        r2 -= w
    for w in tail:
        chunks.append((c0, w))
        c0 += w
    assert c0 == V
    return chunks

CH = 4096  # V-chunk width (2 MiB per streaming DMA; measured best rate)
CHUNKS = _chunk_plan(CH)
NCH = len(CHUNKS)
WARMUP = 1000.0
MOM = 0.9

F32 = mybir.dt.float32
BF16 = mybir.dt.bfloat16
AF = mybir.ActivationFunctionType
ALU = mybir.AluOpType


class _Bacc(bacc.Bacc):
    """Bacc that pins Exp to one ACT table set.

    Only Exp is used; the stock greedy assignment already needs a single
    ACT_TABLE_LOAD, but pinning keeps the choice stable across compiler
    versions.
    """

    def insert_act_table_loads(self):
        from concourse.hw_specs import get_activation_tables

        has_activation = any(
            isinstance(i, mybir.InstActivation)
            for b in self.main_func.blocks
            for i in b.instructions
        )
        if not has_activation:
            return
        tables = []
        for name, fns in get_activation_tables(self.m.arch).items():
            if name != "exp_and_others":
                fns = fns - {AF.Exp}
            tables.append((name, fns))
        import bass_rust

        bass_rust.insert_act_table_loads(self, tables)


def _build(
    ch: int = CH,
    bufs: int = 8,
    exbufs: int = 4,
    split_rings: int = 0,
    fine_tail: int = 1,
) -> bass.Bass:
    # fine_tail=2: group 0 uses the coarse plan (its writeback hides under
    # group 1's stream anyway, so fewer transfers = fewer ring handoffs);
    # only the last group pays for the fine arrival-gated taper.
    if fine_tail == 2:
        plans = [_chunk_plan(ch, 0)] * (NGRP - 1) + [_chunk_plan(ch, 1)]
    else:
        plans = [_chunk_plan(ch, fine_tail)] * NGRP
    nch = max(len(p) for p in plans)
    # Bacc (not raw Bass): its compile pipeline splits multi-semaphore waits
    # into EventSemaphore instructions — TRN2 allows only 1 wait per inst.
    nc = _Bacc("TRN2")
    x = nc.dram_tensor("x", [BLOC, V], F32, kind="ExternalInput")
    out = nc.dram_tensor("out", [BLOC, nch], F32, kind="ExternalOutput")

    with tile.TileContext(nc) as tc:
        with (
            tc.tile_pool(name="stream", bufs=bufs) as stream,
            tc.tile_pool(name="ex", bufs=exbufs) as ex,
            tc.tile_pool(name="small", bufs=1) as small,
        ):
            partials = [
                small.tile([P, len(plans[g])], F32, tag=f"part{g}", name=f"part{g}")
                for g in range(NGRP)
            ]

            # flat chunk schedule across both groups so transfer issue can
            # run ahead of consumption at group boundaries too
            sched = [
                (g, j, c0, w)
                for g in range(NGRP)
                for j, (c0, w) in enumerate(plans[g])
            ]
            pending = {}

            def issue(k):
                g, j, c0, w = sched[k]
                rows = slice(g * P, (g + 1) * P)
                t = stream.tile([P, ch], F32, tag="xt")
                if split_rings:
                    # Half of each transfer on each HWDGE ring (SP + ACT):
                    # the rings drain concurrently through the same 16 SDMA
                    # engines, so per-transfer completion handoffs overlap
                    # instead of gapping the stream. The ACT-ring issue is
                    # emitted one chunk ahead of ACT's exp for that chunk,
                    # so the ACT sequencer never stalls on it.
                    h = (w + 1) // 2
                    nc.sync.dma_start(out=t[:, :h], in_=x[rows, c0 : c0 + h])
                    nc.scalar.dma_start(
                        out=t[:, h:w], in_=x[rows, c0 + h : c0 + w]
                    )
                else:
                    nc.sync.dma_start(out=t[:, :w], in_=x[rows, c0 : c0 + w])
                pending[k] = t

            issue(0)
            for k, (g, j, c0, w) in enumerate(sched):
                rows = slice(g * P, (g + 1) * P)
                gn = len(plans[g])
                last_grp = g == NGRP - 1
                if k + 1 < len(sched):
                    issue(k + 1)
                t = pending.pop(k)
                # exp on ACT (bf16 out — full rate, halves downstream
                # read traffic; ~2^-9 relative rounding is far inside
                # tolerance after the 50k-element sum)
                e_t = ex.tile([P, ch], BF16, tag="et")
                if fine_tail == 2 and last_grp and j == gn - 1:
                    # final chunk: row-sum via ACT's accumulator — skips the
                    # trailing Vector-engine hop entirely.
                    nc.scalar.activation(
                        out=e_t[:, :w],
                        in_=t[:, :w],
                        func=AF.Exp,
                        accum_out=partials[g][:, j : j + 1],
                    )
                    # only the last column remains: [128,1] writeback
                    nc.sync.dma_start(
                        out=out[rows, j : j + 1],
                        in_=partials[g][:, j : j + 1],
                    )
                    continue
                nc.scalar.activation(out=e_t[:, :w], in_=t[:, :w], func=AF.Exp)
                if fine_tail == 2 and last_grp and j == gn - 2:
                    # all-but-last columns: issued now so the descriptor gen
                    # and HBM-write receipt hide under the final chunk's
                    # arrival + exp instead of trailing the kernel. Emitted
                    # before this chunk's fold; it waits on col j's write.
                    pass  # (emitted after the fold below)
                if w % 2 == 0:
                    # fold the two halves together while reducing: DVE
                    # reads w cols but only streams w/2 output cols, so
                    # the row-sum costs half of a plain reduce. The
                    # elementwise sum is scratch — write it in place over
                    # the first half (same-stride elementwise is safe).
                    h = w // 2
                    nc.vector.scalar_tensor_tensor(
                        out=e_t[:, :h],
                        in0=e_t[:, :h],
                        scalar=1.0,
                        in1=e_t[:, h:w],
                        op0=ALU.mult,
                        op1=ALU.add,
                        accum_out=partials[g][:, j : j + 1],
                    )
                else:
                    nc.vector.reduce_sum(
                        out=partials[g][:, j : j + 1],
                        in_=e_t[:, :w],
                        axis=mybir.AxisListType.X,
                    )
                if fine_tail == 2 and last_grp and j == gn - 2:
                    nc.sync.dma_start(
                        out=out[rows, : gn - 1],
                        in_=partials[g][:, : gn - 1],
                    )
                elif j == gn - 1 and not (fine_tail == 2 and last_grp):
                    # group writeback: group 0's is issued mid-stream — on
                    # the scalar HWDGE ring when fine_tail, so its packets
                    # and HBM-write receipt don't stall the sync ring's
                    # stream FIFO; only the last group's small DMA trails
                    # the final chunk.
                    wq = nc.scalar if (fine_tail and g < NGRP - 1) else nc.sync
                    wq.dma_start(
                        out=out[rows, :gn], in_=partials[g][:, :gn]
                    )

    # Run Bacc's compile pipeline (register allocation, event-semaphore
    # splitting) — the PJRT exec path ships the BIR as-is.
    nc.finalize()
    return nc


_NC_CACHE: dict[tuple, bass.Bass] = {}


def _get_nc(cfg: tuple = (CH, 8, 4)) -> bass.Bass:
    if cfg not in _NC_CACHE:
        _NC_CACHE[cfg] = _build(*cfg)
    return _NC_CACHE[cfg]


def run(
    inputs,
    targets,
    sample_ids,
    difficulty_scores,
    step,
    cfg: tuple = (CH, 8, 4),
    **spmd_kwargs,
):
    """Run the SPMD kernel; returns (scalar result, BassKernelResults)."""
    step_i = int(np.asarray(step))
    c = 1.0 - step_i / WARMUP  # curriculum sharpness coefficient
    x = np.ascontiguousarray(np.asarray(inputs, dtype=np.float32))
    t = np.asarray(targets, dtype=np.int64).reshape(B)
    s = np.asarray(sample_ids, dtype=np.int64).reshape(B)
    d = np.asarray(difficulty_scores, dtype=np.float32).reshape(NTAB)

    nc = _get_nc(cfg)
    in_maps = [{"x": x[core * BLOC : (core + 1) * BLOC]} for core in range(NCORES)]
    br = run_bass_kernel_spmd(nc, in_maps, core_ids=list(range(NCORES)), **spmd_kwargs)

    # Host epilogue on the gathered per-chunk row sums: O(B) work.
    parts = np.concatenate(
        [np.asarray(r["out"], dtype=np.float64) for r in br.results], axis=0
    )  # [B, nch_max]
    ft = cfg[4] if len(cfg) > 4 else 1
    if ft == 2:
        # per-group chunk counts differ; unwritten columns are undefined,
        # so sum each row over its own group's column count only
        ncols = [len(_chunk_plan(cfg[0], 0)), len(_chunk_plan(cfg[0], 1))]
        grp = (np.arange(B) % BLOC) // P  # group index of each row
        S = np.where(
            grp == 0,
            parts[:, : ncols[0]].sum(axis=1),
            parts[:, : ncols[1]].sum(axis=1),
        )
    else:
        S = parts.sum(axis=1)  # [B] sum of exps per row
    lse = np.log(S)
    tl = x[np.arange(B), t].astype(np.float64)  # target logits
    base = lse - tl
    new_diff = MOM * d[s].astype(np.float64) + (1.0 - MOM) * base
    e = np.exp(-new_diff * c)
    result = (base * e).sum() / e.sum()  # weight-normalized mean
    return np.asarray(result, dtype=np.float32), br


def kernel(inputs, targets, sample_ids, difficulty_scores, step):
    result, _ = run(inputs, targets, sample_ids, difficulty_scores, step)
    return result
